# revision 1
# baseline (speedup 1.0000x reference)
"""Trainium2 Bass kernel for nn_MicroTransformerLayer.

Sharding: 8 cores = 4 sequences x 2 half-sequence shards. Each core receives
the full sequence's x (transposed, permuted so its own 1024 tokens sit at
context positions 1024:2047), recomputes the full-sequence down-projection
(needed for K/V), and runs attention + FF + up-projection for its own 1024
tokens. The program is SPMD-uniform; per-core differences enter only through
data (a +0/-10000 additive gate bias on the other-half attention scores).

Layout: feature-major (transposed) activations throughout; fp32 storage with
float32r matmuls. Softmax runs without max-subtraction (scores are provably
bounded); the denominator comes from a ones-column appended to V and is
broadcast across partitions with a K=1 outer-product matmul.
"""

import os
import sys

for _p in ("/opt/trn_rl_repo", "/root/.axon_site/_ro/trn_rl_repo"):
    if os.path.isdir(_p) and _p not in sys.path:
        sys.path.append(_p)

import numpy as np

import concourse.bass as bass
import concourse.mybir as mybir
import concourse.tile as tile
from concourse import bacc
from concourse.bass_utils import run_bass_kernel_spmd

F32 = mybir.dt.float32
F32R = mybir.dt.float32r
AF = mybir.ActivationFunctionType

BIG, SMALL, HEADS, HD, FF = 4096, 256, 4, 64, 512
B, T = 4, 2048
SEQ, OWN = 2048, 1024
P, CH = 128, 512
N_CTX_CH = SEQ // CH          # 4
N_OWN_CH = OWN // CH          # 2
KT_BIG = BIG // P             # 32
CTX_TILES = SEQ // P          # 16
EPS = 1.1920929e-07
GATE_OFF = -10000.0
N_CORES = 8


def _emit(nc, tc, d):
    """Emit the per-core program. d: dict of DRAM APs."""
    with (
        tc.tile_pool(name="persist", bufs=1) as pp,
        tc.tile_pool(name="xin", bufs=2) as xin,
        tc.tile_pool(name="hnp", bufs=2) as hnp,
        tc.tile_pool(name="prp", bufs=2) as prp,
        tc.tile_pool(name="work", bufs=1) as wk,
        tc.tile_pool(name="wk2", bufs=2) as wk2,
        tc.tile_pool(name="outp", bufs=2) as outp,
        tc.tile_pool(name="psA", bufs=2, space="PSUM") as psA,
        tc.tile_pool(name="psS", bufs=2, space="PSUM") as psS,
        tc.tile_pool(name="psO", bufs=2, space="PSUM") as psO,
    ):
        # ---- persistent SBUF tensors ----
        w_dd = pp.tile([P, KT_BIG, SMALL], F32R, tag="wbig")      # W_down.T tiles
        for _q in range(4):
            nc.sync.dma_start(
                w_dd[:, bass.ts(_q, 8), :],
                d["wd"].rearrange("(ko ki) m -> ki ko m", ki=P)[:, bass.ts(_q, 8), :],
            )
        w_qkv = pp.tile([P, 2, 3 * SMALL], F32R, tag="wqkv")
        nc.sync.dma_start(w_qkv[:], d["wqkv"].rearrange("(ko ki) m -> ki ko m", ki=P))
        w_o = pp.tile([P, 2, SMALL], F32R, tag="wo")
        nc.sync.dma_start(w_o[:], d["wo"].rearrange("(ko ki) m -> ki ko m", ki=P))
        w_gu = pp.tile([P, 2, 2 * FF], F32R, tag="wgu")
        nc.sync.dma_start(w_gu[:], d["wgu"].rearrange("(ko ki) m -> ki ko m", ki=P))
        w_dff = pp.tile([P, 4, SMALL], F32R, tag="wdff")
        nc.sync.dma_start(w_dff[:], d["wdff"].rearrange("(ko ki) m -> ki ko m", ki=P))
        ones_s = pp.tile([P, P], F32R, tag="ones")
        nc.sync.dma_start(ones_s[:], d["ones"])
        tril_s = pp.tile([P, 896], F32R, tag="tril")
        nc.sync.dma_start(tril_s[:], d["tril"])
        cb_s = pp.tile([P, 2], F32, tag="cbias")   # col0: gate bias, col1: eps
        nc.sync.dma_start(cb_s[:], d["cbias"])

        hT = pp.tile([P, 2, OWN], F32, tag="hT")          # residual stream (own half)
        kT = pp.tile([P, 2, SEQ], F32R, tag="kT")
        qT = pp.tile([P, 2, OWN], F32R, tag="qT")
        vo = pp.tile([P, CTX_TILES, 4 * (HD + 1)], F32R, tag="vo")  # V + ones col per head
        aoT = pp.tile([P, 2, OWN], F32R, tag="aoT")

        # ones columns of vo (65th col of each head block)
        nc.sync.dma_start(
            vo[:].rearrange("p t (h x) -> p t h x", x=HD + 1)[:, :, :, HD : HD + 1],
            d["vones"].rearrange("p (t h) -> p t h", h=4)[:, :, :, None],
        )

        # =============== STAGE A: down-proj + norm1 + QKV over full ctx ========
        for c in range(N_CTX_CH):
            cs = bass.ds(c * CH, CH)
            ph = [psA.tile([P, CH], F32, tag="dn", name=f"ph{_m}") for _m in range(2)]
            for kb in range(KT_BIG // 4):
                xt = xin.tile([P, 4, CH], F32R, tag="xt")
                nc.sync.dma_start(
                    xt[:],
                    d["xT"].rearrange("(ko ki) t -> ki ko t", ki=P)[:, bass.ts(kb, 4), cs],
                )
                for kk in range(4):
                    k = 4 * kb + kk
                    for m in range(2):
                        nc.tensor.matmul(
                            ph[m][:], w_dd[:, k, bass.ts(m, P)], xt[:, kk, :],
                            start=(k == 0), stop=(k == KT_BIG - 1),
                        )
            hch = hnp.tile([P, 2, CH], F32, tag="hch")
            hdst = (lambda m: hT[:, m, bass.ds((c - 2) * CH, CH)]) if c >= 2 else (lambda m: hch[:, m, :])
            hsq = [wk2.tile([P, CH], F32R, tag="hsq", name=f"hsq{_m}") for _m in range(2)]
            for m in range(2):
                nc.vector.tensor_copy(hdst(m), ph[m][:])
                nc.vector.tensor_mul(hsq[m][:], hdst(m), hdst(m))
            pss = psS.tile([P, CH], F32, tag="ps")
            for m in range(2):
                nc.tensor.matmul(pss[:], ones_s[:], hsq[m][:],
                                 start=(m == 0), stop=(m == 1))
            # inv_rms = 1 / sqrt(sumsq/256 + eps)
            lnv = wk.tile([P, CH], F32, tag="lnv")
            nc.scalar.activation(lnv[:], pss[:], AF.Sqrt, scale=1.0 / SMALL, bias=cb_s[:, 1:2])
            rinv = wk.tile([P, CH], F32, tag="rinv")
            nc.vector.reciprocal(rinv[:], lnv[:])
            hn = hnp.tile([P, 2, CH], F32R, tag="hn")
            for m in range(2):
                nc.vector.tensor_mul(hn[:, m, :], hdst(m), rinv[:])
            # K^T (all chunks) and Q^T (own chunks)
            for m in range(2):
                pk = psA.tile([P, CH], F32, tag="dn")
                for kt in range(2):
                    nc.tensor.matmul(pk[:], w_qkv[:, kt, bass.ds(SMALL + m * P, P)],
                                     hn[:, kt, :], start=(kt == 0), stop=(kt == 1))
                nc.vector.tensor_copy(kT[:, m, cs], pk[:])
                if c >= 2:
                    pq = psA.tile([P, CH], F32, tag="dn")
                    for kt in range(2):
                        nc.tensor.matmul(pq[:], w_qkv[:, kt, bass.ds(m * P, P)],
                                         hn[:, kt, :], start=(kt == 0), stop=(kt == 1))
                    nc.vector.tensor_copy(qT[:, m, bass.ds((c - 2) * CH, CH)], pq[:])
            # V token-major, interleaved with per-head ones columns
            for tt in range(4):
                ct = 4 * c + tt
                pv = psA.tile([P, SMALL], F32, tag="dn")
                for kt in range(2):
                    nc.tensor.matmul(pv[:], hn[:, kt, bass.ts(tt, P)],
                                     w_qkv[:, kt, bass.ds(2 * SMALL, SMALL)],
                                     start=(kt == 0), stop=(kt == 1))
                nc.vector.tensor_copy(
                    vo[:, ct, :].rearrange("p (h x) -> p h x", x=HD + 1)[:, :, 0:HD],
                    pv[:].rearrange("p (h x) -> p h x", x=HD),
                )

        # preload W_up.T into the shared big-weight slot (reused after stage A)
        w_up = pp.tile([P, 2, BIG], F32R, tag="wbig")
        for _q in range(4):
            nc.sync.dma_start(
                w_up[:, :, bass.ts(_q, BIG // 4)],
                d["wup"].rearrange("(ko ki) m -> ki ko m", ki=P)[:, :, bass.ts(_q, BIG // 4)],
            )

        # =============== STAGE B: attention ====================================
        for c in range(N_OWN_CH):
            qs = bass.ds(c * CH, CH)
            vis = 8 + 4 * (c + 1)      # visible ctx k-tiles for this q-chunk
            for ft in range(2):        # head pair (2ft, 2ft+1)
                po = [psO.tile([HD + 1, CH], F32, tag="po", name=f"po{_h}") for _h in range(2)]
                for kt in range(vis):
                    ps_s = psS.tile([P, 2 * CH], F32, tag="ps")
                    for hh in range(2):
                        b0 = HD * hh
                        nc.tensor.matmul(
                            ps_s[:, bass.ts(hh, CH)],
                            kT[b0 : b0 + HD, ft, bass.ts(kt, P)],
                            qT[b0 : b0 + HD, ft, qs],
                            start=True, stop=True,
                        )
                    pr = prp.tile([P, 2 * CH], F32R, tag="pr")
                    if kt < 8:
                        nc.scalar.activation(pr[:], ps_s[:], AF.Exp,
                                             bias=cb_s[:, 0:1], scale=0.125)
                    else:
                        nc.scalar.activation(pr[:], ps_s[:], AF.Exp, scale=0.125)
                    di = kt - 8 - 4 * c
                    if 0 <= di <= 3:
                        for hh in range(2):
                            nc.vector.tensor_mul(
                                pr[:, bass.ts(hh, CH)], pr[:, bass.ts(hh, CH)],
                                tril_s[:, bass.ds(384 - 128 * di, CH)],
                            )
                    for hh in range(2):
                        h = 2 * ft + hh
                        nc.tensor.matmul(
                            po[hh][:], vo[:, kt, bass.ts(h, HD + 1)],
                            pr[:, bass.ts(hh, CH)],
                            start=(kt == 0), stop=(kt == vis - 1),
                        )
                for hh in range(2):
                    r64 = wk.tile([P, CH], F32R, tag="r64")
                    nc.vector.tensor_copy(r64[HD : HD + 1, :], po[hh][HD : HD + 1, :])
                    pb = psA.tile([P, CH], F32, tag="dn")
                    nc.tensor.matmul(pb[:], ones_s[HD : HD + 1, 0:P],
                                     r64[HD : HD + 1, :], start=True, stop=True)
                    rb = wk.tile([P, CH], F32, tag="rb")
                    nc.vector.reciprocal(rb[0:HD, :], pb[0:HD, :])
                    nc.vector.tensor_mul(aoT[HD * hh : HD * hh + HD, ft, qs],
                                         po[hh][0:HD, :], rb[0:HD, :])

        # =============== STAGE C: o-proj, norm2, FF, up-proj (own tokens) ======
        for c in range(N_OWN_CH):
            qs = bass.ds(c * CH, CH)                 # own-token slice
            hs = qs                                  # hT holds own half only
            h2 = wk.tile([P, 2, CH], F32, tag="h2")
            for m in range(2):
                pp_ = psA.tile([P, CH], F32, tag="dn")
                for kt in range(2):
                    nc.tensor.matmul(pp_[:], w_o[:, kt, bass.ts(m, P)],
                                     aoT[:, kt, qs], start=(kt == 0), stop=(kt == 1))
                nc.vector.tensor_add(h2[:, m, :], pp_[:], hT[:, m, hs])
            # norm2
            h2sq = [wk2.tile([P, CH], F32R, tag="hsq", name=f"h2sq{_m}") for _m in range(2)]
            for m in range(2):
                nc.vector.tensor_mul(h2sq[m][:], h2[:, m, :], h2[:, m, :])
            pss = psS.tile([P, CH], F32, tag="ps")
            for m in range(2):
                nc.tensor.matmul(pss[:], ones_s[:], h2sq[m][:],
                                 start=(m == 0), stop=(m == 1))
            lnv = wk.tile([P, CH], F32, tag="lnv")
            nc.scalar.activation(lnv[:], pss[:], AF.Sqrt, scale=1.0 / SMALL, bias=cb_s[:, 1:2])
            rinv = wk.tile([P, CH], F32, tag="rinv")
            nc.vector.reciprocal(rinv[:], lnv[:])
            hn2 = wk.tile([P, 2, CH], F32R, tag="hn2")
            for m in range(2):
                nc.vector.tensor_mul(hn2[:, m, :], h2[:, m, :], rinv[:])
            # gate / up FF with exp-based silu (stays in the Ln/Exp table set)
            fT = wk.tile([P, 4, CH], F32R, tag="fT")
            for g in range(4):
                pgate = psA.tile([P, CH], F32, tag="dn")
                for kt in range(2):
                    nc.tensor.matmul(pgate[:], w_gu[:, kt, bass.ts(g, P)],
                                     hn2[:, kt, :], start=(kt == 0), stop=(kt == 1))
                pup = psA.tile([P, CH], F32, tag="dn")
                for kt in range(2):
                    nc.tensor.matmul(pup[:], w_gu[:, kt, bass.ds(FF + g * P, P)],
                                     hn2[:, kt, :], start=(kt == 0), stop=(kt == 1))
                ex = wk.tile([P, CH], F32, tag="ex")
                nc.scalar.activation(ex[:], pgate[:], AF.Exp, scale=-1.0)
                nc.vector.tensor_scalar_add(ex[:], ex[:], 1.0)
                rc = wk.tile([P, CH], F32, tag="rc")
                nc.vector.reciprocal(rc[:], ex[:])
                xs = wk.tile([P, CH], F32, tag="xs")
                nc.vector.tensor_mul(xs[:], pgate[:], rc[:])
                nc.vector.tensor_mul(fT[:, g, :], xs[:], pup[:])
            # ff down + residual
            h3 = wk.tile([P, 2, CH], F32R, tag="h3")
            for m in range(2):
                pf = psA.tile([P, CH], F32, tag="dn")
                for kt in range(4):
                    nc.tensor.matmul(pf[:], w_dff[:, kt, bass.ts(m, P)],
                                     fT[:, kt, :], start=(kt == 0), stop=(kt == 3))
                nc.vector.tensor_add(h3[:, m, :], pf[:], h2[:, m, :])
            # up-projection, streamed out in groups of 4 m-tiles
            for mb in range(KT_BIG // 4):
                yt = outp.tile([P, 4, CH], F32, tag="yt")
                for kk in range(4):
                    m = 4 * mb + kk
                    py = psA.tile([P, CH], F32, tag="dn")
                    for kt in range(2):
                        nc.tensor.matmul(py[:], w_up[:, kt, bass.ts(m, P)],
                                         h3[:, kt, :], start=(kt == 0), stop=(kt == 1))
                    if m % 2 == 0:
                        nc.vector.tensor_copy(yt[:, kk, :], py[:])
                    else:
                        nc.scalar.copy(yt[:, kk, :], py[:])
                nc.sync.dma_start(
                    d["yT"].rearrange("(mo ki) t -> ki mo t", ki=P)[:, bass.ts(mb, 4), qs],
                    yt[:],
                )


def _build():
    nc = bacc.Bacc("TRN2", target_bir_lowering=False, debug=False,
                   num_devices=N_CORES)
    d = {}
    d["xT"] = nc.dram_tensor("xT", [BIG, SEQ], F32R, kind="ExternalInput").ap()
    d["wd"] = nc.dram_tensor("wd", [BIG, SMALL], F32R, kind="ExternalInput").ap()
    d["wqkv"] = nc.dram_tensor("wqkv", [SMALL, 3 * SMALL], F32R, kind="ExternalInput").ap()
    d["wo"] = nc.dram_tensor("wo", [SMALL, SMALL], F32R, kind="ExternalInput").ap()
    d["wgu"] = nc.dram_tensor("wgu", [SMALL, 2 * FF], F32R, kind="ExternalInput").ap()
    d["wdff"] = nc.dram_tensor("wdff", [FF, SMALL], F32R, kind="ExternalInput").ap()
    d["wup"] = nc.dram_tensor("wup", [SMALL, BIG], F32R, kind="ExternalInput").ap()
    d["ones"] = nc.dram_tensor("ones", [P, P], F32R, kind="ExternalInput").ap()
    d["tril"] = nc.dram_tensor("tril", [P, 896], F32R, kind="ExternalInput").ap()
    d["cbias"] = nc.dram_tensor("cbias", [P, 2], F32, kind="ExternalInput").ap()
    d["vones"] = nc.dram_tensor("vones", [P, 64], F32R, kind="ExternalInput").ap()
    d["yT"] = nc.dram_tensor("yT", [BIG, OWN], F32, kind="ExternalOutput").ap()
    with tile.TileContext(nc) as tc:
        _emit(nc, tc, d)
    nc.compile()
    return nc


_NC_CACHE = None


def _get_nc():
    global _NC_CACHE
    if _NC_CACHE is None:
        _NC_CACHE = _build()
    return _NC_CACHE


def make_in_maps(x, W_down, W_up, W_qkv, W_o, W_gate, W_upff, W_downff, g1, g2):
    f32 = np.float32
    shared = {
        "wd": np.ascontiguousarray(W_down.T, dtype=f32),
        "wqkv": np.ascontiguousarray((W_qkv * g1[None, :]).T, dtype=f32),
        "wo": np.ascontiguousarray(W_o.T, dtype=f32),
        "wgu": np.ascontiguousarray(
            np.concatenate([W_gate, W_upff], axis=0).T * 1.0, dtype=f32),
        "wdff": np.ascontiguousarray(W_downff.T, dtype=f32),
        "wup": np.ascontiguousarray(W_up.T, dtype=f32),
        "ones": np.ones((P, P), f32),
        "vones": np.ones((P, 64), f32),
    }
    # fold g2 into the gate/up weights
    shared["wgu"] = np.ascontiguousarray(
        (np.concatenate([W_gate, W_upff], axis=0) * g2[None, :]).T, dtype=f32)
    # tril base: tril[k, j] = 1 if k <= j - 384 else 0  (j in [0,896))
    kk = np.arange(P)[:, None]
    jj = np.arange(896)[None, :]
    shared["tril"] = (kk <= jj - 384).astype(f32)
    in_maps = []
    for b in range(B):
        for j in range(2):
            other = x[b, (1 - j) * OWN : (2 - j) * OWN]
            own = x[b, j * OWN : (j + 1) * OWN]
            xp = np.concatenate([other, own], axis=0)          # [SEQ, BIG]
            m = dict(shared)
            m["xT"] = np.ascontiguousarray(xp.T, dtype=f32)    # [BIG, SEQ]
            cb = np.empty((P, 2), f32)
            cb[:, 0] = 0.0 if j == 1 else GATE_OFF
            cb[:, 1] = EPS
            m["cbias"] = cb
            in_maps.append(m)
    return in_maps


def assemble(results):
    y = np.empty((B, T, BIG), np.float32)
    for b in range(B):
        for j in range(2):
            yT = results[2 * b + j]["yT"]                      # [BIG, OWN]
            y[b, j * OWN : (j + 1) * OWN] = yT.T
    return y


def kernel(x, W_down, W_up, W_qkv, W_o, W_gate, W_upff, W_downff, g1, g2):
    nc = _get_nc()
    in_maps = make_in_maps(x, W_down, W_up, W_qkv, W_o, W_gate, W_upff,
                           W_downff, g1, g2)
    res = run_bass_kernel_spmd(nc, in_maps, core_ids=list(range(N_CORES)))
    return assemble(res.results)

